# revision 13
# baseline (speedup 1.0000x reference)
"""Trainium2 Bass kernel for the CustomDetectionLoss (YOLO-style) problem.

Strategy (data-parallel over batch, 4 images per core x 8 cores):
  host:   slice p3/p4/p5 by batch; build per-image row-major pred [8400, 85]
          (concat of the three scales, matching the reference's N ordering);
          extract the objectness channel; partition GT rows by batch_idx
          ownership (ascending global order, preserving argmax tie-breaks).
  device: per image - top-50 of objectness via two-level max8/match_replace
          extraction, gather the 50 pred rows with indirect DMA, pairwise
          CIoU against the image's compacted GT list, first-index argmax via
          a reverse-iota trick, BCE losses, and per-core partial sums via a
          PE column-sum matmul.
  host:   sum the 8 per-core partial triples and form the 4 output scalars.
"""

import functools

import numpy as np

import concourse.bass as bass
import concourse.mybir as mybir
from concourse.bass import IndirectOffsetOnAxis
from concourse.bass_utils import run_bass_kernel_spmd
from concourse.tile import TileContext

f32 = mybir.dt.float32
i32 = mybir.dt.int32
Act = mybir.ActivationFunctionType
Alu = mybir.AluOpType
AxX = mybir.AxisListType.X

NCORES = 8
B = 32
BPC = B // NCORES          # images per core
N = 8400                   # 80*80 + 40*40 + 20*20
P1, F1 = 112, 75           # 112 * 75 == 8400
K = 50                     # top-k
NCLS = 80
CAND = P1 * 8              # 896 fold candidates per image
NEG = -3.0e38
EPS = 1e-7
C4PI = float(4.0 / np.pi**2)


def _split_excess_waits(nc, max_waits=1):
    """walrus CoreV3 CTRL codegen only tolerates 1 sync-wait per
    instruction; split extra waits onto preceding same-engine drains."""
    for f in nc.m.functions:
        for bb in f.blocks:
            new_list = []
            for inst in bb.instructions:
                si = inst.sync_info
                if si is not None and si.on_wait and len(si.on_wait) > max_waits:
                    waits = list(si.on_wait)
                    keep = waits[:max_waits]
                    overflow = waits[max_waits:]
                    for j in range(0, len(overflow), max_waits):
                        d = mybir.InstDrain(
                            name=f"{inst.name}-wsplit-{j}", ins=[], outs=[]
                        )
                        d.engine = inst.engine
                        d.sync_info = mybir.SyncInfo(
                            on_wait=overflow[j : j + max_waits], on_update=[]
                        )
                        new_list.append(d)
                    inst.sync_info = mybir.SyncInfo(
                        on_wait=keep, on_update=list(si.on_update or [])
                    )
                new_list.append(inst)
            bb.instructions[:] = new_list


def _emit_atan(nc, sb, out_ap, in_ap, shape, tag):
    """Full-range arctan: ACT's Arctan only accepts [-pi/2, pi/2], so use
    atan(x) = sign(x) * (atan(min(|x|, 1/|x|)) + [|x|>1]*(pi/2 - 2*atan(...)))."""
    Act_ = mybir.ActivationFunctionType
    Alu_ = mybir.AluOpType

    def t(n):
        return sb.tile(list(shape), f32, name=f"at_{tag}_{n}")

    ax = t("ax")
    nc.scalar.activation(ax[:, :], in_ap, Act_.Abs)
    inv = t("inv")
    nc.vector.reciprocal(inv[:, :], ax[:, :])
    arg = t("arg")
    nc.vector.tensor_tensor(arg[:, :], ax[:, :], inv[:, :], op=Alu_.min)
    at = t("at")
    nc.scalar.activation(at[:, :], arg[:, :], Act_.Arctan)
    big = t("big")
    nc.vector.tensor_scalar(big[:, :], ax[:, :], 1.0, None, op0=Alu_.is_gt)
    u = t("u")
    nc.scalar.activation(
        u[:, :], at[:, :], Act_.Copy, bias=float(np.pi / 2), scale=-2.0
    )
    vv = t("vv")
    nc.gpsimd.tensor_mul(vv[:, :], big[:, :], u[:, :])
    w = t("w")
    nc.vector.tensor_add(w[:, :], at[:, :], vv[:, :])
    sg = t("sg")
    nc.scalar.activation(sg[:, :], in_ap, Act_.Sign)
    nc.vector.tensor_mul(out_ap, sg[:, :], w[:, :])


@functools.lru_cache(maxsize=4)
def _build(G, debug=False):
    nc = bass.Bass(
        "TRN2", target_bir_lowering=False, debug=debug, num_devices=NCORES
    )
    pred = nc.dram_tensor("pred", [BPC * N, 85], f32, kind="ExternalInput")
    obj = nc.dram_tensor("obj", [BPC, P1, F1], f32, kind="ExternalInput")
    # gtr layout per image: [gx, gy, gw, gh, gval, gcls, hg] each G wide
    gtr = nc.dram_tensor("gtr", [BPC, 7 * G], f32, kind="ExternalInput")
    outp = nc.dram_tensor("out", [3, 1], f32, kind="ExternalOutput")
    vscr = nc.dram_tensor("vscr", [BPC, CAND], f32)
    nscr = nc.dram_tensor("nscr", [BPC, CAND], f32)

    PK = 10 * G + 1  # packed gt quantities + hg

    with TileContext(nc) as tc:
        with (
            tc.tile_pool(name="sb", bufs=1) as sb,
            tc.tile_pool(name="ps", bufs=1, space="PSUM") as ps,
        ):
            # ---- constants ----
            ones50 = sb.tile([K, 1], f32)
            nc.gpsimd.memset(ones50[:, :], 1.0)
            onesT = sb.tile([1, K], f32)
            nc.gpsimd.memset(onesT[:, :], 1.0)
            revi = sb.tile([K, G], i32)
            nc.gpsimd.iota(revi[:, :], [[-1, G]], base=G, channel_multiplier=0)
            revf = sb.tile([K, G], f32)
            nc.vector.tensor_copy(revf[:, :], revi[:, :])
            io80 = sb.tile([K, NCLS], i32)
            nc.gpsimd.iota(io80[:, :], [[1, NCLS]], base=0, channel_multiplier=0)
            io80f = sb.tile([K, NCLS], f32)
            nc.vector.tensor_copy(io80f[:, :], io80[:, :])

            # ---- GT prep: derive per-gt quantities, pack, broadcast ----
            gtall = sb.tile([BPC, 7 * G], f32)
            nc.sync.dma_start(out=gtall[:, :], in_=gtr[:, :])
            gx = gtall[:, 0 * G : 1 * G]
            gy = gtall[:, 1 * G : 2 * G]
            gw = gtall[:, 2 * G : 3 * G]
            gh = gtall[:, 3 * G : 4 * G]
            gval = gtall[:, 4 * G : 5 * G]
            gcls = gtall[:, 5 * G : 6 * G]
            hg = gtall[:, 6 * G : 6 * G + 1]

            gtpack = sb.tile([BPC, PK], f32)
            s_gx1 = gtpack[:, 0 * G : 1 * G]
            s_gx2 = gtpack[:, 1 * G : 2 * G]
            s_gy1 = gtpack[:, 2 * G : 3 * G]
            s_gy2 = gtpack[:, 3 * G : 4 * G]
            s_gxc = gtpack[:, 4 * G : 5 * G]
            s_gyc = gtpack[:, 5 * G : 6 * G]
            s_gae = gtpack[:, 6 * G : 7 * G]
            s_gat = gtpack[:, 7 * G : 8 * G]
            s_gva = gtpack[:, 8 * G : 9 * G]
            s_gcl = gtpack[:, 9 * G : 10 * G]
            s_hg = gtpack[:, 10 * G : 10 * G + 1]

            tw = sb.tile([BPC, G], f32)
            nc.vector.tensor_scalar(tw[:, :], gw, 0.5, None, op0=Alu.mult)
            th = sb.tile([BPC, G], f32)
            nc.vector.tensor_scalar(th[:, :], gh, 0.5, None, op0=Alu.mult)
            nc.vector.tensor_sub(s_gx1, gx, tw[:, :])
            nc.vector.tensor_add(s_gx2, gx, tw[:, :])
            nc.vector.tensor_sub(s_gy1, gy, th[:, :])
            nc.vector.tensor_add(s_gy2, gy, th[:, :])
            nc.gpsimd.tensor_copy(s_gxc, gx)
            nc.gpsimd.tensor_copy(s_gyc, gy)
            ga = sb.tile([BPC, G], f32)
            nc.vector.tensor_mul(ga[:, :], gw, gh)
            nc.vector.tensor_scalar(s_gae, ga[:, :], EPS, None, op0=Alu.add)
            rgh = sb.tile([BPC, G], f32)
            nc.vector.reciprocal(rgh[:, :], gh)
            grat = sb.tile([BPC, G], f32)
            nc.vector.tensor_mul(grat[:, :], gw, rgh[:, :])
            _emit_atan(nc, sb, s_gat, grat[:, :], (BPC, G), "gt")
            nc.gpsimd.tensor_copy(s_gva, gval)
            nc.gpsimd.tensor_copy(s_gcl, gcls)
            nc.gpsimd.tensor_copy(s_hg, hg)

            # broadcast each image's pack row across 50 partitions via PE
            gts = []
            for b in range(BPC):
                pk1 = sb.tile([1, PK], f32, name=f"pk1_{b}")
                nc.sync.dma_start(out=pk1[:, :], in_=gtpack[b : b + 1, :])
                gp_ps = ps.tile([K, PK], f32, name=f"gtps{b}")
                for c0 in range(0, PK, 512):
                    c1 = min(c0 + 512, PK)
                    nc.tensor.matmul(
                        out=gp_ps[:, c0:c1],
                        lhsT=onesT[:, :],
                        rhs=pk1[:, c0:c1],
                        start=True,
                        stop=True,
                    )
                g_sb = sb.tile([K, PK], f32, name=f"gts{b}")
                nc.scalar.copy(g_sb[:, :], gp_ps[:, :])
                gts.append(g_sb)

            # ---- top-k L1: per-partition top-8 of objectness ----
            for b in range(BPC):
                x = sb.tile([P1, F1], f32, name=f"x{b}")
                nc.sync.dma_start(out=x[:, :], in_=obj[b])
                v8 = sb.tile([P1, 8], f32, name=f"v8_{b}")
                nc.vector.max(out=v8[:, :], in_=x[:, :])
                mi = sb.tile([P1, 8], mybir.dt.uint32, name=f"mi{b}")
                nc.vector.max_index(mi[:, :], v8[:, :], x[:, :])
                iop = sb.tile([P1, 8], i32, name=f"iop{b}")
                nc.gpsimd.iota(
                    iop[:, :], [[0, 8]], base=b * N + 1, channel_multiplier=F1
                )
                n1 = sb.tile([P1, 8], i32, name=f"n1_{b}")
                nc.gpsimd.tensor_tensor(
                    n1[:, :], mi[:, :].bitcast(i32), iop[:, :], op=Alu.add
                )
                n1f = sb.tile([P1, 8], f32, name=f"n1f{b}")
                nc.vector.tensor_copy(n1f[:, :], n1[:, :])
                nc.sync.dma_start(
                    out=vscr[b].rearrange("(p f) -> p f", p=P1), in_=v8[:, :]
                )
                nc.sync.dma_start(
                    out=nscr[b].rearrange("(p f) -> p f", p=P1), in_=n1f[:, :]
                )

            vband = sb.tile([BPC, CAND], f32)
            nc.sync.dma_start(out=vband[:, :], in_=vscr[:, :])
            nband = sb.tile([BPC, CAND], f32)
            nc.sync.dma_start(out=nband[:, :], in_=nscr[:, :])

            # ---- top-k L2: extract exactly 50 winners per image ----
            vorig = sb.tile([BPC, CAND], f32)
            nc.scalar.copy(vorig[:, :], vband[:, :])
            for r in range(7):
                g8 = sb.tile([BPC, 8], f32, name=f"g8_{r}")
                nc.vector.max(out=g8[:, :], in_=vband[:, :])
                if r == 6:
                    nc.vector.memset(g8[:, 2:8], NEG)
                nc.vector.match_replace(
                    out=vband[:, :],
                    in_to_replace=g8[:, :],
                    in_values=vband[:, :],
                    imm_value=NEG,
                )
            mask = sb.tile([BPC, CAND], i32)
            nc.vector.tensor_tensor(
                mask[:, :], vband[:, :], vorig[:, :], op=Alu.not_equal
            )
            nsel = sb.tile([BPC, CAND], f32)
            nc.gpsimd.memset(nsel[:, :], -1.0)
            nc.vector.copy_predicated(nsel[:, :], mask[:, :], nband[:, :])

            # ---- compact selected indices, sorted desc, via 7 more rounds ----
            T = sb.tile([32, 64], f32)
            nc.gpsimd.memset(T[:, :], -7.0)
            for r in range(7):
                c8 = T[0:4, 8 * r : 8 * r + 8]
                nc.vector.max(out=c8, in_=nsel[:, :])
                nc.vector.match_replace(
                    out=nsel[:, :],
                    in_to_replace=c8,
                    in_values=nsel[:, :],
                    imm_value=-2.0,
                )
            nT = sb.tile([32, 64], f32)
            nc.vector.transpose(nT[:, :], T[:, :])
            nTi = sb.tile([32, 64], i32)
            nc.vector.tensor_copy(nTi[:, :], nT[:, :])
            # stored values are n+1 (iota base offset for the mask trick)
            nc.vector.tensor_scalar(
                nTi[:, :], nTi[:, :], 1, None, op0=Alu.subtract
            )

            # ---- per-image: gather + CIoU + losses ----
            loss_ps = ps.tile([3, 1], f32)
            for b in range(BPC):
                sel = sb.tile([K, 85], f32, name=f"sel{b}")
                nc.gpsimd.indirect_dma_start(
                    out=sel[0:32, :],
                    out_offset=None,
                    in_=pred[:, :],
                    in_offset=IndirectOffsetOnAxis(
                        ap=nTi[0:32, b : b + 1], axis=0
                    ),
                )
                nc.gpsimd.indirect_dma_start(
                    out=sel[32:K, :],
                    out_offset=None,
                    in_=pred[:, :],
                    in_offset=IndirectOffsetOnAxis(
                        ap=nTi[0 : K - 32, 32 + b : 33 + b], axis=0
                    ),
                )

                px = sel[:, 0:1]
                py = sel[:, 1:2]
                pw = sel[:, 2:3]
                ph = sel[:, 3:4]
                pobj = sel[:, 4:5]
                pcls = sel[:, 5:85]

                def t(name, shape=(K, G)):
                    return sb.tile(list(shape), f32, name=f"{name}{b}")

                # pred-derived scalars [K,1]
                pw2 = t("pw2", (K, 1))
                nc.vector.tensor_scalar(pw2[:, :], pw, 0.5, None, op0=Alu.mult)
                ph2 = t("ph2", (K, 1))
                nc.vector.tensor_scalar(ph2[:, :], ph, 0.5, None, op0=Alu.mult)
                px1 = t("px1", (K, 1))
                nc.vector.tensor_sub(px1[:, :], px, pw2[:, :])
                px2 = t("px2", (K, 1))
                nc.vector.tensor_add(px2[:, :], px, pw2[:, :])
                py1 = t("py1", (K, 1))
                nc.vector.tensor_sub(py1[:, :], py, ph2[:, :])
                py2 = t("py2", (K, 1))
                nc.vector.tensor_add(py2[:, :], py, ph2[:, :])
                parea = t("parea", (K, 1))
                nc.vector.tensor_mul(parea[:, :], pw, ph)
                rph = t("rph", (K, 1))
                nc.vector.reciprocal(rph[:, :], ph)
                prat = t("prat", (K, 1))
                nc.vector.tensor_mul(prat[:, :], pw, rph[:, :])
                patan = t("patan", (K, 1))
                _emit_atan(
                    nc, sb, patan[:, :], prat[:, :], (K, 1), f"p{b}"
                )
                negpx = t("negpx", (K, 1))
                nc.gpsimd.tensor_scalar(
                    negpx[:, :], px, -1.0, None, op0=Alu.mult
                )
                negpy = t("negpy", (K, 1))
                nc.gpsimd.tensor_scalar(
                    negpy[:, :], py, -1.0, None, op0=Alu.mult
                )
                negpat = t("negpat", (K, 1))
                nc.gpsimd.tensor_scalar(
                    negpat[:, :], patan[:, :], -1.0, None, op0=Alu.mult
                )

                g = gts[b]
                GX1 = g[:, 0 * G : 1 * G]
                GX2 = g[:, 1 * G : 2 * G]
                GY1 = g[:, 2 * G : 3 * G]
                GY2 = g[:, 3 * G : 4 * G]
                GXC = g[:, 4 * G : 5 * G]
                GYC = g[:, 5 * G : 6 * G]
                GAE = g[:, 6 * G : 7 * G]
                GAT = g[:, 7 * G : 8 * G]
                GVA = g[:, 8 * G : 9 * G]
                GCL = g[:, 9 * G : 10 * G]
                HG = g[:, 10 * G : 10 * G + 1]

                # intersection
                t1 = t("t1")
                nc.vector.tensor_scalar(t1[:, :], GX2, px2[:, :], None, op0=Alu.min)
                t2 = t("t2")
                nc.vector.tensor_scalar(t2[:, :], GX1, px1[:, :], None, op0=Alu.max)
                iw = t("iw")
                nc.gpsimd.tensor_sub(iw[:, :], t1[:, :], t2[:, :])
                iwc = t("iwc")
                nc.gpsimd.tensor_scalar(iwc[:, :], iw[:, :], 0.0, None, op0=Alu.max)
                t3 = t("t3")
                nc.vector.tensor_scalar(t3[:, :], GY2, py2[:, :], None, op0=Alu.min)
                t4 = t("t4")
                nc.vector.tensor_scalar(t4[:, :], GY1, py1[:, :], None, op0=Alu.max)
                ih = t("ih")
                nc.gpsimd.tensor_sub(ih[:, :], t3[:, :], t4[:, :])
                ihc = t("ihc")
                nc.gpsimd.tensor_scalar(ihc[:, :], ih[:, :], 0.0, None, op0=Alu.max)
                inter = t("inter")
                nc.vector.tensor_mul(inter[:, :], iwc[:, :], ihc[:, :])
                # union & iou
                u1 = t("u1")
                nc.gpsimd.tensor_scalar(u1[:, :], GAE, parea[:, :], None, op0=Alu.add)
                union = t("union")
                nc.vector.tensor_sub(union[:, :], u1[:, :], inter[:, :])
                runion = t("runion")
                nc.vector.reciprocal(runion[:, :], union[:, :])
                iou = t("iou")
                nc.vector.tensor_mul(iou[:, :], inter[:, :], runion[:, :])
                # enclosing box diagonal
                c1 = t("c1")
                nc.gpsimd.tensor_scalar(c1[:, :], GX2, px2[:, :], None, op0=Alu.max)
                c2 = t("c2")
                nc.gpsimd.tensor_scalar(c2[:, :], GX1, px1[:, :], None, op0=Alu.min)
                cw = t("cw")
                nc.gpsimd.tensor_sub(cw[:, :], c1[:, :], c2[:, :])
                c3 = t("c3")
                nc.gpsimd.tensor_scalar(c3[:, :], GY2, py2[:, :], None, op0=Alu.max)
                c4 = t("c4")
                nc.gpsimd.tensor_scalar(c4[:, :], GY1, py1[:, :], None, op0=Alu.min)
                ch = t("ch")
                nc.gpsimd.tensor_sub(ch[:, :], c3[:, :], c4[:, :])
                cw2 = t("cw2")
                nc.scalar.activation(cw2[:, :], cw[:, :], Act.Square)
                ch2 = t("ch2")
                nc.scalar.activation(ch2[:, :], ch[:, :], Act.Square)
                c2s = t("c2s")
                nc.vector.tensor_add(c2s[:, :], cw2[:, :], ch2[:, :])
                c2e = t("c2e")
                nc.gpsimd.tensor_scalar(c2e[:, :], c2s[:, :], EPS, None, op0=Alu.add)
                rc2 = t("rc2")
                nc.vector.reciprocal(rc2[:, :], c2e[:, :])
                # center distance
                dx2 = t("dx2")
                nc.scalar.activation(dx2[:, :], GXC, Act.Square, bias=negpx[:, :])
                dy2 = t("dy2")
                nc.scalar.activation(dy2[:, :], GYC, Act.Square, bias=negpy[:, :])
                rho2 = t("rho2")
                nc.vector.tensor_add(rho2[:, :], dx2[:, :], dy2[:, :])
                # aspect-ratio term
                da2 = t("da2")
                nc.scalar.activation(da2[:, :], GAT, Act.Square, bias=negpat[:, :])
                v = t("v")
                nc.gpsimd.tensor_scalar(v[:, :], da2[:, :], C4PI, None, op0=Alu.mult)
                a1 = t("a1")
                nc.scalar.activation(a1[:, :], iou[:, :], Act.Copy, bias=1.0, scale=-1.0)
                a2 = t("a2")
                nc.vector.tensor_add(a2[:, :], a1[:, :], v[:, :])
                a3 = t("a3")
                nc.gpsimd.tensor_scalar(a3[:, :], a2[:, :], EPS, None, op0=Alu.add)
                ra = t("ra")
                nc.vector.reciprocal(ra[:, :], a3[:, :])
                alpha = t("alpha")
                nc.vector.tensor_mul(alpha[:, :], v[:, :], ra[:, :])
                valpha = t("valpha")
                nc.gpsimd.tensor_mul(valpha[:, :], v[:, :], alpha[:, :])
                pr = t("pr")
                nc.vector.tensor_mul(pr[:, :], rho2[:, :], rc2[:, :])
                pen = t("pen")
                nc.vector.tensor_add(pen[:, :], pr[:, :], valpha[:, :])
                d0 = t("d0")
                nc.vector.tensor_sub(d0[:, :], iou[:, :], pen[:, :])
                dc = t("dc")
                nc.vector.tensor_scalar(
                    dc[:, :], d0[:, :], 0.0, 1.0, op0=Alu.max, op1=Alu.min
                )
                # mask invalid gt slots to exactly -1
                cm1 = t("cm1")
                nc.vector.tensor_mul(cm1[:, :], dc[:, :], GVA)
                cm2 = t("cm2")
                nc.gpsimd.tensor_add(cm2[:, :], cm1[:, :], GVA)
                ciou = t("ciou")
                nc.vector.tensor_scalar(
                    ciou[:, :], cm2[:, :], -1.0, None, op0=Alu.add
                )

                q = sb.tile([K, 3], f32, name=f"q{b}")
                nc.vector.reduce_max(q[:, 0:1], ciou[:, :], axis=AxX)
                eqm = t("eqm")
                nc.vector.tensor_scalar(
                    eqm[:, :], ciou[:, :], q[:, 0:1], None, op0=Alu.is_equal
                )
                junk = t("junk")
                nc.vector.tensor_mul(junk[:, :], eqm[:, :], revf[:, :])
                rmax = t("rmax", (K, 1))
                nc.vector.reduce_max(rmax[:, :], junk[:, :], axis=AxX)
                eq2 = t("eq2")
                nc.vector.tensor_scalar(
                    eq2[:, :], revf[:, :], rmax[:, :], None, op0=Alu.is_equal
                )
                junk2 = t("junk2")
                nc.vector.tensor_mul(junk2[:, :], eq2[:, :], GCL)
                cstar = t("cstar", (K, 1))
                nc.vector.reduce_max(cstar[:, :], junk2[:, :], axis=AxX)

                # losses; softplus(x) = relu(x) + log1p(exp(-|x|))
                oax = t("oax", (K, 1))
                nc.scalar.activation(oax[:, :], pobj, Act.Abs)
                oex = t("oex", (K, 1))
                nc.scalar.activation(oex[:, :], oax[:, :], Act.Exp, scale=-1.0)
                oln = t("oln", (K, 1))
                nc.scalar.activation(oln[:, :], oex[:, :], Act.Ln, bias=1.0)
                orl = t("orl", (K, 1))
                nc.scalar.activation(orl[:, :], pobj, Act.Relu)
                spobj = t("spobj", (K, 1))
                nc.vector.tensor_add(spobj[:, :], oln[:, :], orl[:, :])
                om = t("om", (K, 1))
                nc.gpsimd.tensor_mul(om[:, :], pobj, q[:, 0:1])
                nc.vector.tensor_sub(q[:, 1:2], spobj[:, :], om[:, :])

                cax = t("cax", (K, NCLS))
                nc.scalar.activation(cax[:, :], pcls, Act.Abs)
                cex = t("cex", (K, NCLS))
                nc.scalar.activation(cex[:, :], cax[:, :], Act.Exp, scale=-1.0)
                cln = t("cln", (K, NCLS))
                sumA = t("sumA", (K, 1))
                nc.scalar.activation(
                    cln[:, :], cex[:, :], Act.Ln, bias=1.0, accum_out=sumA[:, :]
                )
                crl = t("crl", (K, NCLS))
                sumB = t("sumB", (K, 1))
                nc.scalar.activation(
                    crl[:, :], pcls, Act.Relu, accum_out=sumB[:, :]
                )
                nc.vector.tensor_add(q[:, 2:3], sumA[:, :], sumB[:, :])
                eqc = t("eqc", (K, NCLS))
                nc.vector.tensor_scalar(
                    eqc[:, :], io80f[:, :], cstar[:, :], None, op0=Alu.is_equal
                )
                junk80 = t("junk80", (K, NCLS))
                nc.vector.tensor_mul(junk80[:, :], eqc[:, :], pcls)
                pick = t("pick", (K, 1))
                nc.vector.reduce_sum(pick[:, :], junk80[:, :], axis=AxX)
                nc.vector.tensor_sub(q[:, 2:3], q[:, 2:3], pick[:, :])
                qs = sb.tile([K, 3], f32, name=f"qs{b}")
                nc.vector.tensor_scalar(
                    qs[:, :], q[:, :], HG, None, op0=Alu.mult
                )
                nc.tensor.matmul(
                    out=loss_ps[:, :],
                    lhsT=qs[:, :],
                    rhs=ones50[:, :],
                    start=(b == 0),
                    stop=(b == BPC - 1),
                )

            out_sb = sb.tile([3, 1], f32)
            nc.scalar.copy(out_sb[:, :], loss_ps[:, :])
            nc.sync.dma_start(out=outp[:, :], in_=out_sb[:, :])

    return nc


@functools.lru_cache(maxsize=4)
def _build_hw(G):
    nc = _build(G)
    _split_excess_waits(nc)
    return nc


def _prep_host(p3, p4, p5, bboxes, cls, batch_idx):
    p3 = np.asarray(p3, dtype=np.float32)
    p4 = np.asarray(p4, dtype=np.float32)
    p5 = np.asarray(p5, dtype=np.float32)
    bboxes = np.asarray(bboxes, dtype=np.float32)
    cls_v = np.asarray(cls).reshape(-1)
    bidx = np.asarray(batch_idx).reshape(-1).astype(np.int64)

    pred = np.ascontiguousarray(
        np.concatenate(
            [p.reshape(B, 85, -1) for p in (p3, p4, p5)], axis=2
        ).transpose(0, 2, 1)
    )  # [B, N, 85]
    obj = np.ascontiguousarray(pred[:, :, 4]).reshape(B, P1, F1)

    counts = np.bincount(bidx, minlength=B)
    G = max(8, int(np.ceil(counts.max() / 4) * 4))

    gtr = np.zeros((B, 7 * G), np.float32)
    H = 0.0
    for b in range(B):
        own = np.where(bidx == b)[0]
        nb = len(own)
        if nb:
            H += 1.0
            gtr[b, 0 * G : 0 * G + nb] = bboxes[own, 0]
            gtr[b, 1 * G : 1 * G + nb] = bboxes[own, 1]
            gtr[b, 2 * G : 2 * G + nb] = bboxes[own, 2]
            gtr[b, 3 * G : 3 * G + nb] = bboxes[own, 3]
            gtr[b, 4 * G : 4 * G + nb] = 1.0
            gtr[b, 5 * G : 5 * G + nb] = cls_v[own].astype(np.float32)
            gtr[b, 6 * G : 7 * G] = 1.0
        # pad boxes stay 0; avoid atan(0/0) by giving pads w=h=0.5
        gtr[b, 2 * G + nb : 3 * G] = 0.5
        gtr[b, 3 * G + nb : 4 * G] = 0.5
    return pred, obj, gtr, G, H


def kernel(p3, p4, p5, bboxes, cls, batch_idx):
    pred, obj, gtr, G, H = _prep_host(p3, p4, p5, bboxes, cls, batch_idx)
    nc = _build_hw(G)
    in_maps = []
    for c in range(NCORES):
        sl = slice(c * BPC, (c + 1) * BPC)
        in_maps.append(
            {
                "pred": np.ascontiguousarray(
                    pred[sl].reshape(BPC * N, 85)
                ),
                "obj": np.ascontiguousarray(obj[sl]),
                "gtr": np.ascontiguousarray(gtr[sl]),
            }
        )
    res = run_bass_kernel_spmd(nc, in_maps, list(range(NCORES)), trace=False)
    S = np.zeros(3, np.float64)
    for c in range(NCORES):
        S += res.results[c]["out"].reshape(3).astype(np.float64)
    loss_box = np.float32((H - S[0] / K) / B)
    loss_obj = np.float32(S[1] / K / B)
    loss_cls = np.float32(S[2] / (K * NCLS) / B)
    total = np.float32(
        0.05 * loss_box + 1.0 * loss_obj + 0.5 * loss_cls
    )
    return np.array([total, loss_box, loss_obj, loss_cls], np.float32)
